# revision 1
# baseline (speedup 1.0000x reference)
"""Correlation layer + softmax(axis=i) Trainium2 kernel.

corr[b,i,j] = sum_c f1[b,c,i] * f2[b,c,j]   (b=4, c=256, i,j in hw=4096)
out = softmax(corr, axis=i) reshaped to (4, 4096, 64, 64)

Sharding: 8 cores = 4 batches x 2 j-halves. Softmax reduces over i, which is
fully local per core when corr is computed transposed (j on partitions, i on
the free axis).

Per core (2048 j x 4096 i), for each of 16 j-tiles (128 j), a chunked
(flash-style) softmax over four 1024-wide i-quarters:
  1. corrT quarter (128 j, 1024 i) = f2_cols.T @ f1 via 4 fp32r matmuls into
     a 2-bank PSUM tile (pool bufs=4 -> all 8 banks, quarters pipeline
     independently),
  2. per-quarter row max m_q (DVE, negated) then exp(corr - m_q) straight off
     PSUM in ONE activation per quarter (amortizes PSUM-access + accumulator
     overhead), per-quarter row sums accumulate on the fly. Using the LOCAL
     quarter max keeps every exp in [0,1] -- overflow-safe for any input and,
     unlike a global row max, never serializes PSUM recycling across quarters.
  3. merge: M = max_q m_q, e_q = exp(m_q - M), S = sum_q sums_q*e_q,
     r_q = e_q / S  (tiny [128,4] ops on DVE + one small ACT exp),
  4. normalize quarter q by r_q -- Pool takes q0..q2, DVE takes q3 (balances
     both engines under the DMA roofline) -- and DMA each quarter out as soon
     as it is scaled.
Input DMAs are chunked in need-order (j-tile-0 cols, f1 halves, remaining f2
cols) so matmuls start ~2us in; output DMA saturates from ~20us on. The
device output is (2048 j, 4096 i) per core; the host transposes during
unsharding (the gather has to copy these bytes anyway).

This walrus build allows only ONE sync wait per instruction. Tile freely
emits several, so kernel.py patches two spots in the Tile pipeline:
  - a post-scheduling pass splits every multi-wait instruction into
    single-wait same-engine Drain carriers ahead of it,
  - the kernel-tail drain (one wait per outstanding semaphore) is split the
    same way.
"""

import sys

import numpy as np

sys.path.insert(0, "/opt/trn_rl_repo")

import concourse.bass as bass
import concourse.mybir as mybir
import concourse.tile as tile
from concourse.bass_utils import run_bass_kernel_spmd

B, C, H, W = 4, 256, 64, 64
HW = H * W  # 4096
JJ = HW // 2  # j columns per core
N_CORES = 8
P = 128
KC = C // P  # 2 contraction chunks
NJT = JJ // P  # 16 j-tiles per core
HB = 2048  # input-load half width
QW = 1024  # softmax quarter width = 2 PSUM banks
NQ = HW // QW  # 4
MMN = 512  # matmul moving width (one PSUM bank)

FP32 = mybir.dt.float32

_split_counter = [0]


def _split_multiwaits(ordered):
    """Walrus (this build) rejects instructions with >1 sync wait. Hoist the
    extra waits onto single-wait Drain instructions on the same engine queue
    immediately before the offender (queues are in-order)."""
    for bb, insts in ordered.items():
        out = []
        changed = False
        for inst in insts:
            si = getattr(inst, "sync_info", None)
            waits = list(si.on_wait) if (si is not None and si.on_wait) else []
            if len(waits) > 1:
                changed = True
                for w in waits[:-1]:
                    _split_counter[0] += 1
                    d = mybir.InstDrain(
                        name=f"I-wsplit-{_split_counter[0]}",
                        ins=[],
                        outs=[],
                        engine=inst.engine,
                    )
                    d.sync_info = mybir.SyncInfo(on_wait=[w], on_update=[])
                    out.append(d)
                si.on_wait = waits[-1:]
            out.append(inst)
        if changed:
            ordered[bb] = out
    return ordered


_orig_postorder = tile.postorder_instruction_blocks


def _patched_postorder(ordered, start_bb_name, postordered):
    _split_multiwaits(ordered)
    return _orig_postorder(ordered, start_bb_name, postordered)


tile.postorder_instruction_blocks = _patched_postorder


def _patched_drain_and_barrier(self, tick_clock, wait_clock):
    """Same single-wait discipline for the kernel-tail drain."""
    from concourse.vector_clock import ScopedClock

    drain_inst = self.nc.sync.drain()
    wait_clock.add_sem_waits(
        drain_inst.ins, ScopedClock({None: tick_clock.global_clock})
    )
    si = drain_inst.ins.sync_info
    waits = list(si.on_wait or []) if si is not None else []
    if len(waits) > 1:
        si.on_wait = waits[:1]
        for w in waits[1:]:
            d2 = self.nc.sync.drain()
            si2 = d2.ins.sync_info
            if si2 is None:
                d2.ins.sync_info = mybir.SyncInfo(on_wait=[w], on_update=[])
            else:
                si2.on_wait = [w]
    self.nc.all_engine_barrier()
    assert self.sems is not None
    popped = self.nc._tile_sem_poison_stack.pop()
    assert popped is self._sem_poison
    self.nc.clear_and_free_semaphores(list(self.sems.allocated().values()))
    self.nc.all_engine_barrier()


tile.TileContext._drain_and_barrier = _patched_drain_and_barrier


def _build_bass():
    nc = bass.Bass()
    mmdt = mybir.dt.float32r
    fj = nc.declare_dram_parameter("fj", [C, JJ], mmdt, isOutput=False)
    fi = nc.declare_dram_parameter("fi", [C, HW], mmdt, isOutput=False)
    out = nc.declare_dram_parameter("out", [JJ, HW], FP32, isOutput=True)

    with tile.TileContext(nc) as tc:
        with (
            tc.tile_pool(name="singles", bufs=1) as singles,
            tc.tile_pool(name="exp", bufs=5) as exp_pool,
            tc.tile_pool(name="stats", bufs=24) as stats,
            tc.tile_pool(name="ps", bufs=4, space="PSUM") as ps_pool,
        ):
            # Input DMAs in need-order. Separate tiles per chunk so a matmul
            # only waits on the chunk it actually reads.
            fj0_sb, fja_sb, fjb_sb, fi_sb = [], [], [], []
            # fj0 on the SWDGE (gpsimd) queue so it issues in parallel with
            # the SP queue's fi stream.
            for cc in range(KC):
                t = singles.tile([P, P], mmdt, tag=f"fj0_{cc}")
                nc.gpsimd.dma_start(out=t, in_=fj[cc * P : (cc + 1) * P, 0:P])
                fj0_sb.append(t)
            for half in range(2):
                for cc in range(KC):
                    t = singles.tile([P, HB], mmdt, tag=f"fi_{half}_{cc}")
                    nc.sync.dma_start(
                        out=t,
                        in_=fi[cc * P : (cc + 1) * P, half * HB : (half + 1) * HB],
                    )
                    fi_sb.append(t)  # index = half*KC + cc
            # all of fi lands first; the fj tail then streams in during tile
            # 0's softmax-merge latency instead of in front of it.
            FJA = 4 * P
            for cc in range(KC):
                t = singles.tile([P, FJA - P], mmdt, tag=f"fja_{cc}")
                nc.sync.dma_start(out=t, in_=fj[cc * P : (cc + 1) * P, P:FJA])
                fja_sb.append(t)
            for cc in range(KC):
                t = singles.tile([P, JJ - FJA], mmdt, tag=f"fjb_{cc}")
                nc.sync.dma_start(out=t, in_=fj[cc * P : (cc + 1) * P, FJA:JJ])
                fjb_sb.append(t)

            def fj_cols(cc, jt):
                if jt == 0:
                    return fj0_sb[cc]
                if jt < 4:
                    return fja_sb[cc][:, (jt - 1) * P : jt * P]
                return fjb_sb[cc][:, (jt - 4) * P : (jt - 3) * P]

            for jt in range(NJT):
                exp_t = exp_pool.tile([P, HW], FP32, tag="exp")
                nm = stats.tile([P, NQ], FP32, tag="nm")  # -m_q
                sums = stats.tile([P, NQ], FP32, tag="sums")
                for q in range(NQ):
                    ps = ps_pool.tile([P, QW], FP32, tag="ps")
                    for sub in range(QW // MMN):
                        ioff = q * QW + sub * MMN
                        half, loff = divmod(ioff, HB)
                        for cc in range(KC):
                            nc.tensor.matmul(
                                ps[:, bass.ts(sub, MMN)],
                                lhsT=fj_cols(cc, jt),
                                rhs=fi_sb[half * KC + cc][:, loff : loff + MMN],
                                start=(cc == 0),
                                stop=(cc == KC - 1),
                            )
                    nc.vector.reduce_max(
                        out=nm[:, q : q + 1],
                        in_=ps,
                        axis=mybir.AxisListType.X,
                        negate=True,
                    )
                    nc.scalar.activation(
                        out=exp_t[:, q * QW : (q + 1) * QW],
                        in_=ps,
                        func=mybir.ActivationFunctionType.Exp,
                        bias=nm[:, q : q + 1],
                        scale=1.0,
                        accum_out=sums[:, q : q + 1],
                    )
                # merge: M = max_q m_q; e_q = exp(m_q - M); S = sum sums_q*e_q
                nM = stats.tile([P, 1], FP32, tag="nM")  # -M
                nc.vector.tensor_reduce(
                    out=nM, in_=nm, axis=mybir.AxisListType.X, op=mybir.AluOpType.min
                )
                eq = stats.tile([P, NQ], FP32, tag="eq")
                nc.scalar.activation(
                    out=eq,
                    in_=nm,
                    func=mybir.ActivationFunctionType.Exp,
                    bias=nM,
                    scale=-1.0,
                )
                w = stats.tile([P, NQ], FP32, tag="w")
                nc.vector.tensor_mul(w, sums, eq)
                S = stats.tile([P, 1], FP32, tag="S")
                nc.vector.reduce_sum(out=S, in_=w, axis=mybir.AxisListType.X)
                rs = stats.tile([P, 1], FP32, tag="rs")
                nc.vector.reciprocal(out=rs, in_=S)
                r = stats.tile([P, NQ], FP32, tag="r")
                nc.vector.tensor_scalar_mul(r, eq, rs)
                # normalize + stream out: DVE takes q3 (it owns r, finishes
                # first), Pool then q0..q2 -- DMAs ladder out in that order.
                nc.vector.tensor_scalar_mul(
                    exp_t[:, 3 * QW : 4 * QW], exp_t[:, 3 * QW : 4 * QW], r[:, 3:4]
                )
                nc.sync.dma_start(
                    out=out[jt * P : (jt + 1) * P, 3 * QW : 4 * QW],
                    in_=exp_t[:, 3 * QW : 4 * QW],
                )
                for q in range(3):
                    nc.gpsimd.tensor_scalar_mul(
                        exp_t[:, q * QW : (q + 1) * QW],
                        exp_t[:, q * QW : (q + 1) * QW],
                        r[:, q : q + 1],
                    )
                    nc.sync.dma_start(
                        out=out[jt * P : (jt + 1) * P, q * QW : (q + 1) * QW],
                        in_=exp_t[:, q * QW : (q + 1) * QW],
                    )
    return nc


_NC = None


def _get_nc():
    global _NC
    if _NC is None:
        _NC = _build_bass()
    return _NC


def _run(feat1, feat2, trace=False):
    f1 = np.asarray(feat1, dtype=np.float32).reshape(B, C, HW)
    f2 = np.asarray(feat2, dtype=np.float32).reshape(B, C, HW)
    in_maps = []
    for d in range(N_CORES):
        bb, jh = d // 2, d % 2
        in_maps.append(
            {
                "fj": np.ascontiguousarray(f2[bb][:, jh * JJ : (jh + 1) * JJ]),
                "fi": np.ascontiguousarray(f1[bb]),
            }
        )
    res = run_bass_kernel_spmd(_get_nc(), in_maps, list(range(N_CORES)), trace=trace)
    out = np.empty((B, HW, HW), np.float32)
    for d in range(N_CORES):
        bb, jh = d // 2, d % 2
        # device tile is (j_local, i); transpose during unshard
        out[bb][:, jh * JJ : (jh + 1) * JJ] = res.results[d]["out"].T
    return out.reshape(B, HW, H, W), res


def kernel(feat1, feat2):
    out, _ = _run(feat1, feat2)
    return out



# revision 6
# speedup vs baseline: 1.3457x; 1.3457x over previous
"""Correlation layer + softmax(axis=i) Trainium2 kernel, v14.

corr[b,i,j] = sum_c f1[b,c,i] * f2[b,c,j]   (b=4, c=256, i,j in hw=4096)
out = softmax(corr, axis=i) reshaped to (4, 4096, 64, 64)

Sharding: 8 cores = 4 batches x 2 j-halves. Softmax reduces over i, fully
local per core with corr computed transposed (j on partitions, i free).

The baseline was DMA-device-bound (~39.5MB/core fp32 IO at the shared
360GB/s DMA device = ~110us). v4 changes:
  1. host downcasts inputs to fp16 (matmul still 1 cycle/row; input DMA
     6MB -> 3MB),
  2. exp + output in bf16 (output DMA 33.5MB -> 16.8MB; host upcasts
     during the gather/transpose it already does),
  3. flash-style softmax with per-half (2048) local max: corr here spans
     +-245 with row maxes as low as 51, so no fixed bias fits fp32-exp
     range; the local max keeps every exp in (0,1].
  4. engine assignment tuned against the cost model:
     - per-half max on DVE via tensor_tensor_reduce [256|256] pairs per
       512-sub (op0=max, scale=-1, op1=min => accumulates -max), with the
       accumulator chained across the 4 subs so no separate merge reduce,
     - exp 2048-wide on ACT (the only exp engine) with fp32 accum row
       sums; ACT is the critical engine at ~2.17us/half,
     - merge across halves: nM=min (DVE), e=exp(nM-nm) tiny ACT,
       S=sum(s*e) one DVE ttr, rs=1/S (DVE),
     - normalize on Pool: bf16 tensor_scalar two-scalar (x*e_h)*rs,
     - output DMA per half from SP (HWDGE).
  5. ONE manually ring-buffered [128,4096] PSUM tile (all 8 banks) instead
     of pool-rotated tiles: the Tile scheduler tracks PSUM deps at slice
     level within a tile, so the DVE max-reads of half h overlap the
     matmuls of half h+1; pool-tile rotation serialized reader vs next
     writes at whole-tile granularity (measured 3623 vs 2337 ns/half).
  6. eight warmup matmuls on zero tiles before the input DMAs land, so the
     PE p-state ramp (0.65->2.4GHz over 3us) overlaps the DMA fill.

This walrus build allows only ONE sync wait per instruction; two Tile
pipeline patches below split multi-wait instructions into single-wait Drain
carriers (same as the baseline).
"""

import sys

import numpy as np

sys.path.insert(0, "/opt/trn_rl_repo")

import concourse.bass as bass
import concourse.mybir as mybir
import concourse.tile as tile
from concourse.bass_utils import run_bass_kernel_spmd

B, C, H, W = 4, 256, 64, 64
HW = H * W  # 4096
JJ = HW // 2  # j columns per core
N_CORES = 8
P = 128
KC = C // P  # 2 contraction chunks
NJT = JJ // P  # 16 j-tiles per core
HALF = 2048  # softmax chunk width = 4 PSUM banks
NH = HW // HALF  # 2
MMN = 512  # matmul moving width (one PSUM bank)
NSUB = HALF // MMN  # 4

FP32 = mybir.dt.float32
FP16 = mybir.dt.float16
BF16 = mybir.dt.bfloat16

_split_counter = [0]


def _split_multiwaits(ordered):
    """Walrus (this build) rejects instructions with >1 sync wait. Hoist the
    extra waits onto single-wait Drain instructions on the same engine queue
    immediately before the offender (queues are in-order)."""
    for bb, insts in ordered.items():
        out = []
        changed = False
        for inst in insts:
            si = getattr(inst, "sync_info", None)
            waits = list(si.on_wait) if (si is not None and si.on_wait) else []
            if len(waits) > 1:
                changed = True
                for w in waits[:-1]:
                    _split_counter[0] += 1
                    d = mybir.InstDrain(
                        name=f"I-wsplit-{_split_counter[0]}",
                        ins=[],
                        outs=[],
                        engine=inst.engine,
                    )
                    d.sync_info = mybir.SyncInfo(on_wait=[w], on_update=[])
                    out.append(d)
                si.on_wait = waits[-1:]
            out.append(inst)
        if changed:
            ordered[bb] = out
    return ordered


_orig_postorder = tile.postorder_instruction_blocks


def _patched_postorder(ordered, start_bb_name, postordered):
    _split_multiwaits(ordered)
    return _orig_postorder(ordered, start_bb_name, postordered)


tile.postorder_instruction_blocks = _patched_postorder


def _patched_drain_and_barrier(self, tick_clock, wait_clock):
    """Same single-wait discipline for the kernel-tail drain."""
    from concourse.vector_clock import ScopedClock

    drain_inst = self.nc.sync.drain()
    wait_clock.add_sem_waits(
        drain_inst.ins, ScopedClock({None: tick_clock.global_clock})
    )
    si = drain_inst.ins.sync_info
    waits = list(si.on_wait or []) if si is not None else []
    if len(waits) > 1:
        si.on_wait = waits[:1]
        for w in waits[1:]:
            d2 = self.nc.sync.drain()
            si2 = d2.ins.sync_info
            if si2 is None:
                d2.ins.sync_info = mybir.SyncInfo(on_wait=[w], on_update=[])
            else:
                si2.on_wait = [w]
    self.nc.all_engine_barrier()
    assert self.sems is not None
    popped = self.nc._tile_sem_poison_stack.pop()
    assert popped is self._sem_poison
    self.nc.clear_and_free_semaphores(list(self.sems.allocated().values()))
    self.nc.all_engine_barrier()


tile.TileContext._drain_and_barrier = _patched_drain_and_barrier


def _build_bass():
    nc = bass.Bass()
    fj = nc.declare_dram_parameter("fj", [C, JJ], FP16, isOutput=False)
    fi = nc.declare_dram_parameter("fi", [C, HW], FP16, isOutput=False)
    out = nc.declare_dram_parameter("out", [JJ, HW], BF16, isOutput=True)

    with tile.TileContext(nc) as tc:
        with (
            tc.tile_pool(name="singles", bufs=1) as singles,
            tc.tile_pool(name="exp", bufs=3) as exp_pool,
            tc.tile_pool(name="stats", bufs=24) as stats,
            tc.tile_pool(name="psb", bufs=1, space="PSUM") as psb,
        ):
            # whole-PSUM ring buffer; halves alternate the two 2048 ranges
            ps = psb.tile([P, HW], FP32, tag="ps")

            # p-state warmup: zero matmuls keep the PE busy from ~1us so the
            # 0.65->2.4GHz ramp overlaps the input-DMA fill. memsets go on
            # DVE so the gpsimd queue starts its fj0 SWDGE immediately.
            wl = singles.tile([P, P], FP16, tag="wl")
            wr = singles.tile([P, MMN], FP16, tag="wr")
            nc.vector.memset(wl, 0.0)
            nc.vector.memset(wr, 0.0)
            for k in range(8):
                nc.tensor.matmul(
                    ps[:, HALF + (k % NSUB) * MMN : HALF + (k % NSUB + 1) * MMN],
                    lhsT=wl,
                    rhs=wr,
                    start=True,
                    stop=True,
                )

            # Input DMAs in need-order, finer chunks first so matmul 0 can
            # start as early as possible. fj0 rides the SWDGE (gpsimd) queue,
            # parallel to the SP queue's fi stream.
            fj0_sb, fjr_sb = [], []
            for cc in range(KC):
                t = singles.tile([P, P], FP16, tag=f"fj0_{cc}")
                nc.gpsimd.dma_start(out=t, in_=fj[cc * P : (cc + 1) * P, 0:P])
                fj0_sb.append(t)
            # fi chunk plan: (offset, width) in need-order; chunks are
            # independent tiles so a matmul only waits on what it reads.
            FICHUNKS = [(0, 512), (512, 512), (1024, 1024), (2048, 1024), (3072, 1024)]
            fi_tiles = {}  # (cc, offset) -> (tile, width)
            for off, wdt in FICHUNKS:
                for cc in range(KC):
                    t = singles.tile(
                        [P, wdt], FP16, tag=f"fi_{cc}_{off}", name=f"fi_{cc}_{off}"
                    )
                    fi_tiles[(cc, off)] = (t, wdt)

            def emit_fi_dma(off):
                for cc in range(KC):
                    t, wdt = fi_tiles[(cc, off)]
                    nc.sync.dma_start(
                        out=t, in_=fi[cc * P : (cc + 1) * P, off : off + wdt]
                    )

            for off, _ in FICHUNKS[:3]:
                emit_fi_dma(off)

            def fi_slice(cc, ioff):
                for off, wdt in FICHUNKS:
                    if off <= ioff < off + wdt:
                        t, _ = fi_tiles[(cc, off)]
                        return t[:, ioff - off : ioff - off + MMN]
                raise AssertionError(ioff)

            # fj tail in two pieces: tiles 1-4 arrive early (between fi
            # chunks), the rest after fi. j-tile 1 would otherwise stall on
            # the whole tail.
            FJA = 5 * P  # fj cols [P, FJA) arrive early
            fja_sb = []
            for cc in range(KC):
                t = singles.tile([P, FJA - P], FP16, tag=f"fja_{cc}")
                nc.sync.dma_start(out=t, in_=fj[cc * P : (cc + 1) * P, P:FJA])
                fja_sb.append(t)
            for off, _ in FICHUNKS[3:]:
                emit_fi_dma(off)
            for cc in range(KC):
                t = singles.tile([P, JJ - FJA], FP16, tag=f"fjb_{cc}")
                nc.sync.dma_start(out=t, in_=fj[cc * P : (cc + 1) * P, FJA:JJ])
                fjr_sb.append(t)

            def fj_cols(cc, jt):
                if jt == 0:
                    return fj0_sb[cc]
                if jt < FJA // P:
                    return fja_sb[cc][:, (jt - 1) * P : jt * P]
                return fjr_sb[cc][:, (jt - FJA // P) * P : (jt - FJA // P + 1) * P]

            def emit_mm_chunk(jt, k, start_i, width, exp_t, nm, s, ring):
                """matmuls + one DVE reduce_max + exp for one chunk.

                HW rules: only DVE can reduce along the free axis, GPSIMD
                cannot touch PSUM, and no engine op may read two non-scalar
                PSUM inputs. So the per-chunk max is ONE plain DVE
                reduce_max over the chunk's PSUM range (negate=True gives
                -max for the exp bias). It runs while the PE fills the next
                ring slot; the 3-slot ring {1536,1536,1024} gives the
                mm -> reduce -> exp chain enough depth to pipeline.
                """
                # uniform plans: ring slot k is the chunk's own offset, so
                # tile t+1's chunk k WARs only tile t's chunk k (3-deep ring)
                r = start_i
                nsub = width // MMN
                for sub in range(nsub):
                    ioff = start_i + sub * MMN
                    for cc in range(KC):
                        nc.tensor.matmul(
                            ps[:, r + sub * MMN : r + (sub + 1) * MMN],
                            lhsT=fj_cols(cc, jt),
                            rhs=fi_slice(cc, ioff),
                            start=(cc == 0),
                            stop=(cc == KC - 1),
                        )
                nc.vector.reduce_max(
                    out=nm[:, k : k + 1],
                    in_=ps[:, r : r + width],
                    axis=mybir.AxisListType.X,
                    negate=True,
                )
                nc.scalar.activation(
                    out=exp_t[:, start_i : start_i + width],
                    in_=ps[:, r : r + width],
                    func=mybir.ActivationFunctionType.Exp,
                    bias=nm[:, k : k + 1],
                    scale=1.0,
                    accum_out=s[:, k : k + 1],
                )

            def emit_stage1(jt):
                """nM (Pool ts-min tree) + e (ACT). Emitted 1 tile late,
                at the head of the next tile so both are ready long before
                anything waits on them."""
                plan, exp_t, nm, s = tile_state(jt)
                nch = len(plan)
                st = {}
                nM = stats.tile([P, 1], FP32, tag="nM", name=f"nM_{jt}")
                nc.gpsimd.tensor_scalar(
                    out=nM,
                    in0=nm[:, 0:1],
                    scalar1=nm[:, 1:2],
                    scalar2=None,
                    op0=mybir.AluOpType.min,
                )
                for k in range(2, nch):
                    nc.gpsimd.tensor_scalar(
                        out=nM,
                        in0=nM,
                        scalar1=nm[:, k : k + 1],
                        scalar2=None,
                        op0=mybir.AluOpType.min,
                    )
                e = stats.tile([P, nch], FP32, tag=f"e{nch}", name=f"e_{jt}")
                nc.scalar.activation(
                    out=e,
                    in_=nm,
                    func=mybir.ActivationFunctionType.Exp,
                    bias=nM,
                    scale=-1.0,
                )
                st["e"] = e
                merge_st[jt] = st

            def emit_stage2(jt):
                """w = s*e and S = sum w as per-column Pool tensor_scalar
                ops (walrus allows tensor_scalar but not tensor_tensor on
                Pool); 1/S on DVE."""
                plan, exp_t, nm, s = tile_state(jt)
                nch = len(plan)
                st = merge_st[jt]
                e = st["e"]
                w = stats.tile([P, nch], FP32, tag=f"w{nch}", name=f"w_{jt}")
                for k in range(nch):
                    nc.gpsimd.tensor_scalar(
                        out=w[:, k : k + 1],
                        in0=s[:, k : k + 1],
                        scalar1=e[:, k : k + 1],
                        scalar2=None,
                        op0=mybir.AluOpType.mult,
                    )
                S = stats.tile([P, 1], FP32, tag="S", name=f"S_{jt}")
                nc.gpsimd.tensor_scalar(
                    out=S,
                    in0=w[:, 0:1],
                    scalar1=w[:, 1:2],
                    scalar2=None,
                    op0=mybir.AluOpType.add,
                )
                for k in range(2, nch):
                    nc.gpsimd.tensor_scalar(
                        out=S,
                        in0=S,
                        scalar1=w[:, k : k + 1],
                        scalar2=None,
                        op0=mybir.AluOpType.add,
                    )
                rs = stats.tile([P, 1], FP32, tag="rs", name=f"rs_{jt}")
                nc.vector.reciprocal(out=rs, in_=S)
                st["rs"] = rs

            def emit_stage3(jt, last=False, split=False):
                """r2 = e*rs (Pool), then per-chunk normalize + output DMA.
                Emitted 2 tiles late so every dep is long since resolved."""
                plan, exp_t, nm, s = tile_state(jt)
                nch = len(plan)
                st = merge_st.pop(jt)
                e, rs = st["e"], st["rs"]
                r2 = stats.tile([P, nch], FP32, tag=f"r2{nch}", name=f"r2_{jt}")
                nc.gpsimd.tensor_scalar_mul(r2, e, rs)
                for k in range(nch):
                    start_i, width = plan[k]
                    if last:
                        eng = nc.vector
                    elif split:
                        eng = nc.gpsimd if k % 2 == 0 else nc.vector
                    else:
                        eng = nc.gpsimd
                    eng.tensor_scalar_mul(
                        exp_t[:, start_i : start_i + width],
                        exp_t[:, start_i : start_i + width],
                        r2[:, k : k + 1],
                    )
                    nc.sync.dma_start(
                        out=out[jt * P : (jt + 1) * P, start_i : start_i + width],
                        in_=exp_t[:, start_i : start_i + width],
                    )

            def tile_plan(jt):
                # 3-chunk tiles fill the 8-bank PSUM as a 3-slot ring: deep
                # enough that the lagged DVE reduce pipelines, wide enough
                # that ACT's per-instruction overheads stay amortized
                return [(0, 1536), (1536, 1536), (3072, 1024)]

            merge_st = {}
            ring = [0]
            tiles = {}

            def tile_state(jt):
                if jt not in tiles:
                    plan = tile_plan(jt)
                    nch = len(plan)
                    tiles[jt] = (
                        plan,
                        exp_pool.tile([P, HW], BF16, tag="exp", name=f"exp_{jt}"),
                        stats.tile([P, nch], FP32, tag=f"nm{nch}", name=f"nm_{jt}"),
                        stats.tile([P, nch], FP32, tag=f"s{nch}", name=f"s_{jt}"),
                    )
                return tiles[jt]

            def emit_chunk(jt, k):
                plan, exp_t, nm, s = tile_state(jt)
                start_i, width = plan[k]
                emit_mm_chunk(jt, k, start_i, width, exp_t, nm, s, ring)

            # fill phase: interleave tiles 0 and 1 so the PE has work while
            # the fi tail streams in
            emit_chunk(0, 0)
            emit_chunk(0, 1)
            emit_chunk(1, 0)
            emit_chunk(0, 2)
            emit_chunk(1, 1)
            emit_stage1(0)
            emit_chunk(1, 2)
            emit_stage2(0)
            # steady state: stages of tiles jt-1 / jt-2 interleave with the
            # chunks of tile jt so no in-order queue ever head-of-line-blocks
            for jt in range(2, NJT):
                emit_stage1(jt - 1)
                emit_chunk(jt, 0)
                emit_stage2(jt - 1)
                emit_chunk(jt, 1)
                emit_stage3(jt - 2, split=(jt - 2 == NJT - 2))
                emit_chunk(jt, 2)
            emit_stage1(NJT - 1)
            emit_stage2(NJT - 1)
            emit_stage3(NJT - 2, split=True)
            emit_stage3(NJT - 1, last=True)
    return nc


_NC = None


def _get_nc():
    global _NC
    if _NC is None:
        _NC = _build_bass()
    return _NC


def _run(feat1, feat2, trace=False):
    f1 = np.asarray(feat1, dtype=np.float32).reshape(B, C, HW).astype(np.float16)
    f2 = np.asarray(feat2, dtype=np.float32).reshape(B, C, HW).astype(np.float16)
    in_maps = []
    for d in range(N_CORES):
        bb, jh = d // 2, d % 2
        in_maps.append(
            {
                "fj": np.ascontiguousarray(f2[bb][:, jh * JJ : (jh + 1) * JJ]),
                "fi": np.ascontiguousarray(f1[bb]),
            }
        )
    res = run_bass_kernel_spmd(_get_nc(), in_maps, list(range(N_CORES)), trace=trace)
    out = np.empty((B, HW, HW), np.float32)
    for d in range(N_CORES):
        bb, jh = d // 2, d % 2
        # device tile is (j_local, i) bf16; upcast + transpose during unshard
        out[bb][:, jh * JJ : (jh + 1) * JJ] = (
            np.asarray(res.results[d]["out"]).astype(np.float32).T
        )
    return out.reshape(B, HW, H, W), res


def kernel(feat1, feat2):
    out, _ = _run(feat1, feat2)
    return out
